# revision 22
# baseline (speedup 1.0000x reference)
"""Causal multi-head attention (B=2, H=16, S=2048, D=64, fp32 I/O) on 8 TRN2
NeuronCores.

Sharding: batch*heads (32 units) split 4-per-core — embarrassingly parallel,
no collectives.

Per-core kernel design (bf16 compute, fp32 PSUM accumulation):
  - scores are computed TRANSPOSED: scoresT[k, q] = K_blk @ Q^T so that the
    softmax numerators P^T[k, q] feed the P@V matmul directly as the moving
    operand (contraction dim k on partitions), with V (natural layout) as the
    stationary operand.
  - A ones-column appended to V accumulates the softmax denominator l[q] in
    the same PSUM accumulation as P@V — no separate reduction pass.
  - exp is fused with the PSUM->SBUF eviction on ScalarE (scale=1/sqrt(D)
    folded into the activation). No max-subtraction: scores ~ N(0,1), no
    overflow risk in fp32 exp.
  - Causal masking: off-diagonal blocks are skipped entirely; diagonal blocks
    get a multiplicative 0/1 upper-triangular mask after exp.
  - q is processed in halves of 1024 so PSUM fits: out^T[65,1024] (2 banks,
    double-buffered) + scoresT slots [128,1024] (2 banks, double-buffered).
  - Epilogue: copy out^T to SBUF bf16, DMA-transpose back to natural layout
    (the l column rides along), reciprocal + per-partition scale, f32 out.
  - Q^T/K^T layouts are built with bf16 DMA-transposes; K^T lands directly in
    a "paired slab" layout (kj even on partitions 0:64, kj odd on 64:128) so
    consecutive kj matmuls use disjoint PE row-groups and overlap in-array.
"""

import numpy as np

import concourse.bass as bass
import concourse.mybir as mybir
import concourse.tile as tile
from concourse import bacc
from concourse.bass_utils import run_bass_kernel_spmd
from concourse.masks import make_upper_triangular

B, H, S, D = 2, 16, 2048, 64
N_CORES = 8
HPC = (B * H) // N_CORES  # heads per core
NT = S // 128  # 16 k/q blocks of 128
FP32 = mybir.dt.float32
BF16 = mybir.dt.bfloat16


def build_attention():
    nc = bacc.Bacc("TRN2", target_bir_lowering=False)
    q_d = nc.dram_tensor("query", [HPC, S, D], FP32, kind="ExternalInput")
    k_d = nc.dram_tensor("key", [HPC, S, D], FP32, kind="ExternalInput")
    v_d = nc.dram_tensor("value", [HPC, S, D], FP32, kind="ExternalInput")
    o_d = nc.dram_tensor("out", [HPC, S, D], FP32, kind="ExternalOutput")

    with tile.TileContext(nc) as tc:
        with (
            tc.tile_pool(name="singles", bufs=1) as singles,
            tc.tile_pool(name="nat", bufs=4) as nat_pool,
            tc.tile_pool(name="bf", bufs=3) as bf_pool,
            tc.tile_pool(name="slab", bufs=2) as slab_pool,
            tc.tile_pool(name="qt", bufs=2) as qt_pool,
            tc.tile_pool(name="pt", bufs=5) as pt_pool,
            tc.tile_pool(name="ep", bufs=3) as ep_pool,
            tc.tile_pool(name="sc", bufs=3, space="PSUM") as sc_pool,
            tc.tile_pool(name="ops", bufs=1, space="PSUM") as ops_pool,
        ):
            # 0/1 mask, keep k <= q (partition = k, free = q)
            tri01 = singles.tile([128, 128], BF16, tag="tri01")
            make_upper_triangular(nc, tri01, val=1.0, diag=True)

            for h in range(HPC):
                # ---- load + cast + transpose setup ----
                natQ = nat_pool.tile([128, NT, D], FP32, tag="natQ")
                natK = nat_pool.tile([128, NT, D], FP32, tag="natK")
                natV = nat_pool.tile([128, NT, D], FP32, tag="natV")
                qsrc = q_d[h].rearrange("(t p) d -> p t d", p=128)
                ksrc = k_d[h].rearrange("(t p) d -> p t d", p=128)
                vsrc = v_d[h].rearrange("(t p) d -> p t d", p=128)
                # halved loads: the first q/k halves reach the PE sooner
                hm = NT // 2
                nc.sync.dma_start(out=natQ[:, 0:hm, :], in_=qsrc[:, 0:hm, :])
                nc.sync.dma_start(out=natK[:, 0:hm, :], in_=ksrc[:, 0:hm, :])
                nc.sync.dma_start(out=natQ[:, hm:NT, :], in_=qsrc[:, hm:NT, :])
                nc.sync.dma_start(out=natK[:, hm:NT, :], in_=ksrc[:, hm:NT, :])

                # bfQ2 duplicates each 64-col d-block: chunk j = [Q_j | Q_j],
                # so its 128-wide transpose lands Q^T_j on BOTH partition
                # halves (the odd-kj matmuls read rows 64:128) with no
                # SBUF->SBUF rearrange DMAs.
                bfQ2 = bf_pool.tile([128, NT, 2, D], BF16, tag="bfQ2")
                bfK = bf_pool.tile([128, NT, D], BF16, tag="bfK")
                vaug = bf_pool.tile([128, NT, D + 1], BF16, tag="vaug")
                for sl in (slice(0, hm), slice(hm, NT)):
                    nc.vector.tensor_copy(bfQ2[:, sl, 0, :], natQ[:, sl, :])
                    nc.vector.tensor_copy(bfQ2[:, sl, 1, :], natQ[:, sl, :])
                    nc.vector.tensor_copy(bfK[:, sl, :], natK[:, sl, :])
                # V loads/casts after Q/K: the PV matmuls lag a full pair
                # behind, so V is off the startup critical path
                nc.sync.dma_start(out=natV, in_=vsrc)
                nc.vector.tensor_copy(vaug[:, :, 0:D], natV)
                nc.vector.memset(vaug[:, :, D : D + 1], 1.0)

                # K paired slabs: rows 0:64 = K^T_{2j}, rows 64:128 = K^T_{2j+1}
                # (one blocked DMA-transpose: each 128-col chunk of the input
                # is transposed into out[:, j, :])
                kslab = slab_pool.tile([128, NT // 2, 128], BF16, tag="kslab")
                qt3 = qt_pool.tile([128, NT, 128], BF16, tag="qt")
                bfK_f = bfK.rearrange("p t d -> p (t d)")
                bfQ2_f = bfQ2.rearrange("p t c d -> p (t c d)")
                for j0, j1 in ((0, hm // 2), (hm // 2, NT // 2)):
                    nc.sync.dma_start_transpose(
                        out=kslab[:, j0:j1, :], in_=bfK_f[:, j0 * 128 : j1 * 128]
                    )
                    nc.sync.dma_start_transpose(
                        out=qt3[:, 2 * j0 : 2 * j1, :],
                        in_=bfQ2_f[:, j0 * 256 : j1 * 256],
                    )
                # qt[p, 128j + q'] = Q^T, q-contiguous, identical on both
                # partition halves (from the duplicated bfQ2 chunks)
                qt = qt3.rearrange("p t i -> p (t i)")

                # ---- main loop: q halves x k blocks ----
                for hf in range(2):
                    q0 = 1024 * hf  # absolute start of this q-half
                    q1 = q0 + 1024
                    kj_hi = 8 * (hf + 1)  # kj in [0, kj_hi)
                    # last kj writing each 512-bank of out^T (for stop flags)
                    last_kj = [
                        max(
                            kj
                            for kj in range(kj_hi)
                            if max(q0, 128 * kj) < q0 + 512 * (b + 1)
                        )
                        for b in range(2)
                    ]

                    outps = ops_pool.tile([80, 2, 512], FP32, tag="outps")
                    outps_f = outps.rearrange("p a b -> p (a b)")

                    # kj processed in even/odd pairs: the two QK^T matmuls use
                    # disjoint PE row-groups (partitions 0:64 vs 64:128) and
                    # run concurrently in-array. The PV matmuls are
                    # software-pipelined one pair behind: the PE queue is
                    # strictly in-order, so a PV issued right after its exp
                    # would head-of-line-block the next pair's independent
                    # QK matmuls while waiting on ScalarE.
                    def emit_pv(pair, qas, chunks):
                        # lane-outer: one V_aug weight load per kj; matmuls
                        # split on the absolute 512 grid (PSUM bank limit)
                        for lane, (kj, qa) in enumerate(zip(pair, qas)):
                            for ca, cb, ptile in chunks:
                                lo = max(ca, qa)
                                while lo < cb:
                                    hi = min(cb, q0 + 512 * ((lo - q0) // 512 + 1))
                                    b = (lo - q0) // 512
                                    nc.tensor.matmul(
                                        outps_f[0:65, lo - q0 : hi - q0],
                                        vaug[:, kj, :],
                                        ptile[:, lane, lo - ca : hi - ca],
                                        start=(kj == 0),
                                        stop=(kj == last_kj[b]),
                                    )
                                    lo = hi

                    pending = []
                    for pj in range(kj_hi // 2):
                        pair = (2 * pj, 2 * pj + 1)
                        qas = [max(q0, 128 * kj) for kj in pair]
                        # Both lanes of a pair share one PSUM slot and one exp:
                        # the two QK^T matmuls then become ready together (same
                        # WAR release) and execute concurrently in disjoint PE
                        # row-groups.
                        chunks = []
                        for ca in range(qas[0], q1, 512):
                            cb = min(ca + 512, q1)
                            cols = cb - ca
                            slot = sc_pool.tile(
                                [128, 2, 512], FP32, tag="slot", name="slot"
                            )
                            for lane, (kj, qa) in enumerate(zip(pair, qas)):
                                lo = max(ca, qa)
                                if lo >= cb:
                                    continue
                                rows = (kj % 2) * 64
                                nc.tensor.matmul(
                                    slot[:, lane, lo - ca : cols],
                                    kslab[rows : rows + 64, kj // 2, :],
                                    qt[rows : rows + 64, lo:cb],
                                    start=True,
                                    stop=True,
                                )
                            ptile = pt_pool.tile(
                                [128, 2, 512], BF16, tag="ptile", name="ptile"
                            )
                            # the odd lane's first 128 cols in its diagonal
                            # chunk exp stale PSUM; PV never reads them
                            nc.scalar.activation(
                                ptile[:, :, 0:cols],
                                slot[:, :, 0:cols],
                                mybir.ActivationFunctionType.Exp,
                                scale=1.0 / np.sqrt(D),
                            )
                            for lane, (kj, qa) in enumerate(zip(pair, qas)):
                                dg = 128 * kj
                                if ca <= dg < cb:  # diagonal block in chunk
                                    nc.vector.tensor_mul(
                                        ptile[:, lane, dg - ca : dg - ca + 128],
                                        ptile[:, lane, dg - ca : dg - ca + 128],
                                        tri01,
                                    )
                            chunks.append((ca, cb, ptile))
                        pending.append((pair, qas, chunks))
                        if pj >= 1:
                            emit_pv(*pending.pop(0))
                    for args in pending:
                        emit_pv(*args)

                    # ---- epilogue for this (head, half) ----
                    # rows 65:80 copy PSUM garbage; they transpose into
                    # columns 65:80 of onat which are never read
                    bfo = ep_pool.tile([80, 1024], BF16, tag="bfo")
                    nc.vector.tensor_copy(bfo, outps_f[0:80, :])
                    onat = ep_pool.tile([128, 8, 80], BF16, tag="onat")
                    nc.sync.dma_start_transpose(out=onat, in_=bfo)
                    rec = ep_pool.tile([128, 8], FP32, tag="rec")
                    nc.vector.reciprocal(rec, onat[:, :, D])
                    fo = ep_pool.tile([128, 8, D], FP32, tag="fo")
                    for t in range(8):
                        nc.vector.tensor_scalar_mul(
                            fo[:, t, :], onat[:, t, 0:D], rec[:, t : t + 1]
                        )
                    odst = o_d[h].rearrange("(t p) d -> p t d", p=128)
                    nc.sync.dma_start(out=odst[:, 8 * hf : 8 * hf + 8, :], in_=fo)

    nc.compile()
    return nc


_NC = None


def _get_nc():
    global _NC
    if _NC is None:
        _NC = build_attention()
    return _NC


def kernel(query, key, value):
    nc = _get_nc()
    q = np.ascontiguousarray(query, dtype=np.float32).reshape(B * H, S, D)
    k = np.ascontiguousarray(key, dtype=np.float32).reshape(B * H, S, D)
    v = np.ascontiguousarray(value, dtype=np.float32).reshape(B * H, S, D)
    in_maps = [
        {
            "query": q[i * HPC : (i + 1) * HPC],
            "key": k[i * HPC : (i + 1) * HPC],
            "value": v[i * HPC : (i + 1) * HPC],
        }
        for i in range(N_CORES)
    ]
    res = run_bass_kernel_spmd(nc, in_maps, core_ids=list(range(N_CORES)))
    out = np.concatenate([res.results[i]["out"] for i in range(N_CORES)], axis=0)
    return out.reshape(B, H, S, D)
